# revision 21
# baseline (speedup 1.0000x reference)
"""Trainium2 Bass kernel for nn_DQN_9904194584789 (GNN message passing DQN).

Reference math (B=16, A=256, U=64, T=3):
    cur_sol = x[:,0,:]; mask = x[:,1,:]; w = x[:,2:,:]          # [B,A,A]
    adj = (w != 0)
    e1 = cur_sol[:,:,None] @ W0                                  # rank-1
    e3 = (sum_j relu(w[...,None] * W3) / A) @ W2                 # collapses:
         sum_j relu(w*c) = P*relu(c) + N*relu(-c) elementwise in c, with
         P = sum_j relu(w), N = P - S, S = sum_j w, T = sum_j |w|
         P = (T+S)/2, N = (T-S)/2  =>  e3 expressible from any 2 of {P,S,T}
         with per-column coefficients folded into the F2 stationary.
    base = e1 + e3 (loop invariant);  emb_1 = relu(base)
    emb_{t+1} = relu(base + (adj @ emb_t / A) @ W1)
    heads: dueling MLP on emb_T, row-sum corrections, + 10*mask.

Sharding: pure data-parallel over batch B (2 batches per core x 8 cores).

Fast path (w has no exact zeros -> adj all-ones, checked on host): the
kernel is latency-bound (fixed ~9.7us NEFF startup+teardown + ~1.1us w
stream); the fast path minimizes the serial dependency chain after the
last w byte lands:
  - w streams as 4 [128,256] tiles on the two HWDGE queues (sync=b0,
    scalar=b1), packed weights follow the w tiles on the same queues so
    they never delay the stream; gpsimd only casts cur_sol and builds
    the identity/constants early.
  - the 8 row-sum reductions (2 per tile) are spread across all three
    EW engines with mixed column types: vector tensor_reduce(|w|) -> T
    (no dump write), scalar activation-accum (Copy -> S, Relu -> P),
    gpsimd tensor_scalar-accum -> S.  Per-column e3 coefficients are
    folded into two per-half F2 stationaries host-side.
  - PE transposes each Gc half as soon as its 4 columns land; vector
    copies PSUM->SBUF in its idle window; base = F2.T @ G2 runs as two
    half matmuls ordered so neither blocks the other's inputs.
  - message passing: colsum-accum (vector) -> c matmul (hi/lo fp16)
    -> STT relu-accum -> c2 -> EMB, all PSUM-resident.
  - heads: h1 relu is split across vector/scalar column halves; the
    v-head runs on scalar behind its h1 half; the advantage row-sum and
    v-dot accumulate into one PSUM cell (pkb) interleaved with pa.
  - out = pa + pkb + (10*mask + kc) is one STT + one DMA on sync.

General path (exact zeros present) is the faithful per-batch adjacency
version, unchanged from the previous revision.

Precision: fp16 operands for all matmuls except the tiny v/ra dots
(f32).  Row sums accumulate in f32, stored f16.  ~2e-3 scale-relative
error.
"""

import numpy as np
from contextlib import ExitStack

import concourse.bass as bass
import concourse.bacc as bacc
import concourse.tile as tile
from concourse import mybir
from concourse.bass_utils import run_bass_kernel_spmd
from concourse.masks import make_identity

f32 = mybir.dt.float32
f16 = mybir.dt.float16
Alu = mybir.AluOpType
Act = mybir.ActivationFunctionType
AxX = mybir.AxisListType.X

B, A, U, HID = 16, 256, 64, 64
NCORES = 8
BPC = B // NCORES  # batches per core
INV_A = 1.0 / A

WEIGHT_NAMES = [
    "W0", "W1", "W2", "W3",
    "a_w1", "a_b1", "a_w2", "a_b2", "a_w3", "a_b3",
    "v_w1", "v_b1", "v_w2", "v_b2",
]


def _place_f16(wp: np.ndarray, row0: int, col: int, arr: np.ndarray):
    """Pack fp16 bits of arr pairwise into f32 columns of wp."""
    raw = np.ascontiguousarray(
        np.asarray(arr, np.float32).astype(np.float16)
    ).view(np.uint16)
    k = raw.shape[1]
    pad = np.zeros((raw.shape[0], (k + 1) // 2 * 2), np.uint16)
    pad[:, :k] = raw
    fview = pad.view(np.float32)
    wp[row0:row0 + fview.shape[0], col:col + fview.shape[1]] = fview


# ---------------------------------------------------------------------------
# FAST PATH (no exact zeros in w): merged-batch stacked-partition kernel
# ---------------------------------------------------------------------------
# wp2 [128, NW2] f32 column layout (host-packed, streamed on the gpsimd
# queue in two chunks: the early-needed F2/bias columns first, the bulk
# matmul weights after, so the HWDGE queues carry only w tiles):
FW_F2 = 0      # fp16 F2 stationary (both halves)    [6, 128] -> 64 cols
FW_AW3 = 64    # fp16 stacked a_w3                   [64, 2] -> 1 col
FW_VW2 = 65    # f32 stacked v_w2                    [128, 2]
FW_AB1 = 67    # f32 [a_b1; a_b1]                    [128, 1]
FW_VB1 = 68    # f32 [v_b1; v_b1]                    [128, 1]
FW_AB2 = 69    # f32 [a_b2; a_b2]                    [64, 1]
FW_KC = 70     # f32 kc = A*v_b2 - (A-1)*a_b3        [2, 1]
FW_AW3F = 71   # f32 stacked -a_w3 (negated for psv-ra accum) [64, 2]
FW_W1H = 73    # fp16 hi(block_diag(W1, W1)/A)       [128, 128] -> 64 cols
FW_W1L = 137   # fp16 lo residual of same            [128, 128] -> 64 cols
FW_AW1 = 201   # fp16 block_diag(a_w1)               [128, 128] -> 64 cols
FW_VW1 = 265   # fp16 block_diag(v_w1)               [128, 128] -> 64 cols
FW_AW2 = 329   # fp16 block_diag(a_w2)               [128, 64] -> 32 cols
NW2 = 361
WPA = 73       # early chunk = cols [0, WPA)

# Gc column layout: per t-half [T_b0, S_b0, T_b1, S_b1] (T = sum|w| on
# vector, S = sum w on vector or scalar); identical types both halves
# so a single F2 stationary serves both base matmuls.


def _pack_weights_fast(inputs: dict) -> np.ndarray:
    wp = np.zeros((128, NW2), dtype=np.float32)

    def bd(m):
        z = np.zeros_like(m)
        return np.block([[m, z], [z, m]])

    W1 = np.asarray(inputs["W1"], np.float32)
    w1a = bd(W1) * INV_A
    w1h = w1a.astype(np.float16).astype(np.float32)
    _place_f16(wp, 0, FW_W1H, w1a)
    _place_f16(wp, 0, FW_W1L, w1a - w1h)

    W3 = np.asarray(inputs["W3"], np.float32)[0]          # [64]
    W2m = np.asarray(inputs["W2"], np.float32)            # [64, 64]
    rp = np.maximum(W3, 0.0) @ W2m                        # [64]
    rn = np.maximum(-W3, 0.0) @ W2m
    w0 = np.asarray(inputs["W0"], np.float32)[0]          # [64]
    # e3 = T*(rp+rn)/(2A) + S*(rp-rn)/(2A) with T = sum|w|, S = sum w
    cT = (rp + rn) * (0.5 * INV_A)
    cS = (rp - rn) * (0.5 * INV_A)
    F2 = np.zeros((6, 128), np.float32)
    F2[0, 0:64] = cT
    F2[1, 0:64] = cS
    F2[2, 64:128] = cT
    F2[3, 64:128] = cS
    F2[4, 0:64] = w0
    F2[5, 64:128] = w0
    _place_f16(wp, 0, FW_F2, F2)

    _place_f16(wp, 0, FW_AW1, bd(np.asarray(inputs["a_w1"], np.float32)))
    _place_f16(wp, 0, FW_VW1, bd(np.asarray(inputs["v_w1"], np.float32)))
    _place_f16(wp, 0, FW_AW2, bd(np.asarray(inputs["a_w2"], np.float32)))
    aw3 = np.asarray(inputs["a_w3"], np.float32)[:, 0]    # [32]
    A3 = np.zeros((64, 2), np.float32)
    A3[0:32, 0] = aw3
    A3[32:64, 1] = aw3
    _place_f16(wp, 0, FW_AW3, A3)
    v2 = np.asarray(inputs["v_w2"], np.float32)[:, 0]     # [64]
    wp[0:64, FW_VW2] = v2
    wp[64:128, FW_VW2 + 1] = v2
    ab1 = np.asarray(inputs["a_b1"], np.float32)
    vb1 = np.asarray(inputs["v_b1"], np.float32)
    ab2 = np.asarray(inputs["a_b2"], np.float32)
    wp[:, FW_AB1] = np.concatenate([ab1, ab1])
    wp[:, FW_VB1] = np.concatenate([vb1, vb1])
    wp[0:64, FW_AB2] = np.concatenate([ab2, ab2])
    kc = float(A) * float(np.asarray(inputs["v_b2"])[0]) \
        - float(A - 1) * float(np.asarray(inputs["a_b3"])[0])
    wp[0:2, FW_KC] = kc
    wp[0:64, FW_AW3F:FW_AW3F + 2] = -A3
    return wp


def _build_fast() -> bass.Bass:
    nc = bacc.Bacc(
        "TRN2", target_bir_lowering=False, debug=False, num_devices=NCORES
    )
    xs = nc.declare_dram_parameter("xs", [BPC, A + 2, A], f32, isOutput=False)
    wpd = nc.declare_dram_parameter("wp2", [128, NW2], f32, isOutput=False)
    out = nc.declare_dram_parameter("out", [BPC, A], f32, isOutput=True)

    with tile.TileContext(nc) as tc, ExitStack() as ctx:
        cp = ctx.enter_context(tc.tile_pool(name="const", bufs=1))
        sp = ctx.enter_context(tc.tile_pool(name="scratch", bufs=2))

        # ---------- input DMAs ------------------------------------------
        # w tiles: k index = [b0t0, b0t1, b1t0, b1t1].  The two HWDGE
        # queues carry exactly two w tiles each (1KB descriptors); wp
        # goes via gpsimd software DGE as ONE full-width-row DMA (1.4KB
        # descriptors - column-chunking would shrink descriptors and
        # descriptor count, not bytes, is what the DMA engines charge).
        wt4 = cp.tile([128, 4, 256], f32, tag="wt4")
        wp = cp.tile([128, NW2], f32, tag="wp")
        nc.sync.dma_start(wt4[:, 0, :], xs[0, 2:130, :])
        nc.sync.dma_start(wt4[:, 1, :], xs[0, 130:258, :])
        nc.scalar.dma_start(wt4[:, 2, :], xs[1, 2:130, :])
        nc.scalar.dma_start(wt4[:, 3, :], xs[1, 130:258, :])
        # wp rows 0:6 carry F2 (needed mid-chain); the bulk rows are
        # deferred below so they do not steal stream bandwidth from w.
        nc.gpsimd.dma_start(wp[0:6, :], wpd[0:6, :])

        # gpsimd: cur_sol cast + identity + zeros; mask on sync (tiny)
        G2 = cp.tile([6, 256], f16, tag="G2")
        nc.gpsimd.dma_start(G2[4:6, :], xs[:, 0, :])
        mrowf = cp.tile([2, 256], f32, tag="mrowf")
        nc.sync.dma_start(mrowf[:], xs[:, 1, :])
        ident = cp.tile([128, 128], f16, tag="ident")
        make_identity(nc, ident[:])
        zeros = cp.tile([128, 256], f32, tag="zeros")
        nc.gpsimd.memset(zeros[:], 0.0)
        # m10k = mask * 10 + kc ... computed after the bulk wp arrives
        nc.gpsimd.dma_start(wp[6:128, :], wpd[6:128, :])

        # fp16 weight views
        F2v = wp[0:6, FW_F2:FW_F2 + 64].bitcast(f16)        # [6, 128]
        aw1v = wp[:, FW_AW1:FW_AW1 + 64].bitcast(f16)       # [128, 128]
        vw1v = wp[:, FW_VW1:FW_VW1 + 64].bitcast(f16)       # [128, 128]
        aw2v = wp[:, FW_AW2:FW_AW2 + 32].bitcast(f16)       # [128, 64]
        aw3v = wp[0:64, FW_AW3:FW_AW3 + 1].bitcast(f16)     # [64, 2]
        vw2v = wp[:, FW_VW2:FW_VW2 + 2]                     # [128, 2] f32
        w1hv = wp[:, FW_W1H:FW_W1H + 64].bitcast(f16)       # [128, 128]
        w1lv = wp[:, FW_W1L:FW_W1L + 64].bitcast(f16)       # [128, 128]

        with tc.tile_pool(name="pg", bufs=2, space="PSUM") as pg, \
             tc.tile_pool(name="pb", bufs=1, space="PSUM") as pb, \
             tc.tile_pool(name="pc", bufs=1, space="PSUM") as pc, \
             tc.tile_pool(name="ph", bufs=2, space="PSUM") as ph, \
             tc.tile_pool(name="pf", bufs=1, space="PSUM") as pf:

            # ---- row-sum reductions into Gc [128, 8] f16 ---------------
            # col k semantics per GC_TYPES_*; engines: V=tensor_reduce,
            # Sc=activation-accum, G=tensor_scalar-accum.
            Gc = cp.tile([128, 8], f16, tag="Gc")

            with nc.allow_low_precision(reason="f16 Gc accum, f32 acc HW"):
                # vector: T sums as TWO strided multi-accumulate reduces
                # (t0 tiles then t1 tiles; the ~240ns fixed op overhead
                # amortizes across the pair).  scalar: three S sums.
                nc.vector.tensor_reduce(
                    Gc[:, 0:3:2], wt4[:, 0:3:2, :], axis=AxX, op=Alu.add,
                    apply_absolute_value=True,
                )
                nc.vector.tensor_reduce(
                    Gc[:, 4:8:2], wt4[:, 1:4:2, :], axis=AxX, op=Alu.add,
                    apply_absolute_value=True,
                )
                dsc0 = sp.tile([128, 256], f16, tag="dsc")
                nc.scalar.activation(
                    dsc0[:], wt4[:, 0, :], Act.Copy, accum_out=Gc[:, 1:2]
                )
                dsc1 = sp.tile([128, 256], f16, tag="dsc")
                nc.scalar.activation(
                    dsc1[:], wt4[:, 2, :], Act.Copy, accum_out=Gc[:, 3:4]
                )
                dsc2 = sp.tile([128, 256], f16, tag="dsc")
                nc.scalar.activation(
                    dsc2[:], wt4[:, 3, :], Act.Copy, accum_out=Gc[:, 7:8]
                )

                # ---- per-half transpose -> G2, pipelined with the tail
                # of the reductions (copies on vector: 118ns vs 358)
                ps_base = pb.tile([128, 256], f32, tag="psbase")
                psT0 = pg.tile([4, 128], f16, tag="psT")
                nc.tensor.transpose(psT0[:], Gc[:, 0:4], ident[:])
                nc.vector.tensor_copy(G2[0:4, 0:128], psT0[:])
                nc.tensor.matmul(ps_base[:, 0:128], F2v, G2[:, 0:128])
                nc.vector.tensor_reduce(
                    Gc[:, 5:6], wt4[:, 1, :], axis=AxX, op=Alu.add,
                )
                psT1 = pg.tile([4, 128], f16, tag="psT")
                nc.tensor.transpose(psT1[:], Gc[:, 4:8], ident[:])
                nc.vector.tensor_copy(G2[0:4, 128:256], psT1[:])
            nc.tensor.matmul(ps_base[:, 128:256], F2v, G2[:, 128:256])

            # m10k = mask * 10 + kc (gpsimd, off-chain)
            m10f = cp.tile([2, 256], f32, tag="m10f")
            nc.gpsimd.tensor_scalar(
                m10f[:], mrowf[:], 10.0, wp[0:2, FW_KC:FW_KC + 1],
                Alu.mult, Alu.add,
            )

            # ---- message passing: 2 bias updates off colsums -----------
            csc = cp.tile([128, 2], f16, tag="csc")
            d0 = sp.tile([128, 256], f16, tag="dump")
            nc.vector.tensor_scalar(
                d0[:], ps_base[:], 0.0, None, Alu.max, op1=Alu.add,
                accum_out=csc[:, 0:1],
            )
            ps_c1 = pc.tile([128, 1], f32, tag="psc")
            nc.tensor.matmul(ps_c1[:], w1hv, csc[:, 0:1], start=True,
                             stop=False)
            nc.tensor.matmul(ps_c1[:], w1lv, csc[:, 0:1], start=False,
                             stop=True)

            d1 = sp.tile([128, 256], f16, tag="dump")
            nc.vector.scalar_tensor_tensor(
                d1[:], ps_base[:], ps_c1[:, 0:1], zeros[:],
                Alu.add, Alu.max, accum_out=csc[:, 1:2],
            )
            ps_c2 = pc.tile([128, 1], f32, tag="psc")
            nc.tensor.matmul(ps_c2[:], w1hv, csc[:, 1:2], start=True,
                             stop=False)
            nc.tensor.matmul(ps_c2[:], w1lv, csc[:, 1:2], start=False,
                             stop=True)

            EMB = cp.tile([128, 256], f16, tag="EMB")
            nc.vector.tensor_scalar(
                EMB[:], ps_base[:], ps_c2[:, 0:1], 0.0, Alu.add, op1=Alu.max
            )

            # ---- dueling heads (block-diagonal, both batches at once) ---
            ph1 = ph.tile([128, 256], f32, tag="pmat")
            nc.tensor.matmul(ph1[:], aw1v, EMB[:])
            phv = ph.tile([128, 256], f32, tag="pmat")
            nc.tensor.matmul(phv[:], vw1v, EMB[:])

            h1 = cp.tile([128, 256], f16, tag="h1")
            nc.vector.tensor_scalar(
                h1[:], ph1[:], wp[:, FW_AB1:FW_AB1 + 1], 0.0,
                Alu.add, op1=Alu.max,
            )
            # v-head entirely on scalar (its only post-reduce job)
            hvd = sp.tile([128, 256], f16, tag="hvd")
            hvcs = sp.tile([128, 1], f32, tag="hvcs")
            nc.scalar.activation(
                hvd[:], phv[:], Act.Relu, bias=wp[:, FW_VB1:FW_VB1 + 1],
                accum_out=hvcs[:],
            )

            ph2 = ph.tile([64, 256], f32, tag="pmat")
            nc.tensor.matmul(ph2[:], aw2v, h1[:])
            h2 = sp.tile([64, 256], f16, tag="h2")
            h2cs = sp.tile([64, 1], f32, tag="h2cs")
            nc.vector.scalar_tensor_tensor(
                h2[:], ph2[:], wp[0:64, FW_AB2:FW_AB2 + 1], zeros[0:64, :],
                Alu.add, Alu.max, accum_out=h2cs[:],
            )

            pkb = pf.tile([2, 1], f32, tag="pkb")
            nc.tensor.matmul(pkb[:], vw2v, hvcs[:], start=True, stop=False)
            pa = pf.tile([2, 256], f32, tag="pa")
            nc.tensor.matmul(pa[:], aw3v, h2[:])
            # pkb += (-a_w3bd)^T @ h2cs  ->  pkb = psv - ra
            nc.tensor.matmul(pkb[:], wp[0:64, FW_AW3F:FW_AW3F + 2], h2cs[:],
                             start=False, stop=True)

            # out = pa + [psv - ra] + (10*mask + kc)
            FIN = cp.tile([2, 256], f32, tag="FIN")
            nc.vector.scalar_tensor_tensor(
                FIN[:], pa[:], pkb[:, 0:1], m10f[:], Alu.add, Alu.add
            )
            nc.sync.dma_start(out[:, :], FIN[:])

    return nc


# ---------------------------------------------------------------------------
# GENERAL PATH (exact zeros in w): faithful per-batch adjacency matmuls
# (unchanged baseline implementation)
# ---------------------------------------------------------------------------
# wpack [64, NWP] f32 column layout (host-packed replicated params, one DMA).
WP_W1 = 0          # [64, 64] f32
WP_W2 = 64         # [64, 64] f32
WP_W3 = 128        # [64, 1] f32 column
WP_AB1 = 129       # [64, 1] f32
WP_VB1 = 130       # [64, 1] f32
WP_AB2 = 131       # [32, 1] f32 (padded)
WP_VW2 = 132       # [64, 1] f32
WP_AB3 = 133       # scalar at [0, 133]
WP_VB2 = 134       # scalar at [0, 134]
WP_AW1H = 135      # [64, 64] fp16 -> 32 f32 cols
WP_AW2H = 167      # [64, 32] fp16 -> 16 f32 cols
WP_AW3H = 183      # [32, 1] fp16 padded -> 1 f32 col
WP_VW1H = 184      # [64, 64] fp16 -> 32 f32 cols
WP_W0C = 216       # [64, 1] fp16 column (W0 transposed) -> 1 f32 col
WP_W2H = 217       # [64, 64] fp16 -> 32 f32 cols
NWP = 249


def _pack_weights(inputs: dict) -> np.ndarray:
    wp = np.zeros((64, NWP), dtype=np.float32)
    wp[:, WP_W1:WP_W1 + 64] = inputs["W1"]
    wp[:, WP_W2:WP_W2 + 64] = inputs["W2"]
    wp[:, WP_W3] = inputs["W3"][0]
    wp[:, WP_AB1] = inputs["a_b1"]
    wp[:, WP_VB1] = inputs["v_b1"]
    wp[:32, WP_AB2] = inputs["a_b2"]
    wp[:, WP_VW2] = inputs["v_w2"][:, 0]
    wp[0, WP_AB3] = inputs["a_b3"][0]
    wp[0, WP_VB2] = inputs["v_b2"][0]

    _place_f16(wp, 0, WP_AW1H, inputs["a_w1"])
    _place_f16(wp, 0, WP_AW2H, inputs["a_w2"])
    _place_f16(wp, 0, WP_AW3H, np.asarray(inputs["a_w3"])[:, 0:1])
    _place_f16(wp, 0, WP_VW1H, inputs["v_w1"])
    _place_f16(wp, 0, WP_W0C, np.asarray(inputs["W0"]).T)   # [64, 1]
    _place_f16(wp, 0, WP_W2H, inputs["W2"])
    return wp


def _build_general() -> bass.Bass:
    nc = bacc.Bacc(
        "TRN2", target_bir_lowering=False, debug=False, num_devices=NCORES
    )
    xs = nc.declare_dram_parameter("xs", [BPC, A + 2, A], f32, isOutput=False)
    wpd = nc.declare_dram_parameter("wpack", [64, NWP], f32, isOutput=False)
    out = nc.declare_dram_parameter("out", [BPC, A], f32, isOutput=True)

    with tile.TileContext(nc) as tc, ExitStack() as ctx:
        cp = ctx.enter_context(tc.tile_pool(name="const", bufs=1))
        sp = ctx.enter_context(tc.tile_pool(name="scratch", bufs=2))

        wp = cp.tile([64, NWP], f32, tag="wp")
        nc.sync.dma_start(wp[:], wpd[:])
        wt4 = cp.tile([128, 2 * BPC, A], f32, tag="wt4")
        for b in range(BPC):
            nc.scalar.dma_start(
                wt4[:, 2 * b: 2 * b + 2, :],
                xs[b, 2: A + 2, :].rearrange("(t p) j -> p t j", p=128),
            )
        csc = cp.tile([128, 2 * BPC], f32, tag="csc")
        for b in range(BPC):
            nc.gpsimd.dma_start(
                csc[:, 2 * b: 2 * b + 2],
                xs[b, 0, :].rearrange("(t p) -> p t", p=128),
            )
        mrow = cp.tile([1, BPC * A], f32, tag="mrow")
        nc.gpsimd.dma_start(
            mrow[:].rearrange("p (b a) -> p b a", b=BPC),
            xs[:, 1, :][None, :, :],
        )

        aw1h = wp[:, WP_AW1H:WP_AW1H + 32].bitcast(f16)
        aw2h = wp[:, WP_AW2H:WP_AW2H + 16].bitcast(f16)
        aw3h = wp[0:32, WP_AW3H:WP_AW3H + 1].bitcast(f16)[:, 0:1]
        vw1h = wp[:, WP_VW1H:WP_VW1H + 32].bitcast(f16)
        w0c = wp[:, WP_W0C:WP_W0C + 1].bitcast(f16)[:, 0:1]

        ident = cp.tile([128, 128], f16, tag="ident")
        make_identity(nc, ident[:])
        identf = cp.tile([128, 128], f32, tag="identf")
        make_identity(nc, identf[:])

        with tc.tile_pool(name="psetup", bufs=2, space="PSUM") as psetup:
            w2h = wp[:, WP_W2H:WP_W2H + 32].bitcast(f16)
            w3p = cp.tile([U, 1], f16, tag="w3p")
            nc.scalar.activation(w3p[:], wp[:, WP_W3:WP_W3 + 1], Act.Relu)
            w3n = cp.tile([U, 1], f16, tag="w3n")
            nc.scalar.activation(w3n[:], wp[:, WP_W3:WP_W3 + 1], Act.Relu,
                                 scale=-1.0)
            Fc = cp.tile([U, 3], f16, tag="Fc")
            nc.vector.tensor_copy(Fc[:, 0:1], w0c)
            pspc = psetup.tile([U, 1], f32, tag="pscol")
            nc.tensor.matmul(pspc[:], w2h, w3p[:])
            nc.scalar.mul(Fc[:, 1:2], pspc[:], INV_A)
            psnc = psetup.tile([U, 1], f32, tag="pscol")
            nc.tensor.matmul(psnc[:], w2h, w3n[:])
            nc.scalar.mul(Fc[:, 2:3], psnc[:], INV_A)
            psF = psetup.tile([3, U], f16, tag="psF")
            nc.tensor.transpose(psF[:], Fc[:], ident[0:U, 0:U])
            F = cp.tile([3, U], f16, tag="F")
            nc.vector.tensor_copy(F[:], psF[:])

        t256 = cp.tile([1, 1], f32, tag="t256")
        nc.gpsimd.tensor_scalar(
            t256[:], wp[0:1, WP_VB2:WP_VB2 + 1], float(A), None, Alu.mult
        )
        kc = cp.tile([1, 1], f32, tag="kc")
        nc.gpsimd.tensor_scalar(
            kc[:], wp[0:1, WP_AB3:WP_AB3 + 1], -float(A - 1), t256[:],
            Alu.mult, Alu.add,
        )

        m10 = cp.tile([1, BPC * A], f32, tag="m10")
        nc.scalar.mul(m10[:], mrow[:], 10.0)

        FIN = cp.tile([1, BPC * A], f32, tag="FIN")

        with tc.tile_pool(name="pmm", bufs=1, space="PSUM") as pmm, \
             tc.tile_pool(name="pbase", bufs=2, space="PSUM") as pbase, \
             tc.tile_pool(name="phead", bufs=2, space="PSUM") as phead:
            for b in range(BPC):
                Tb = sp.tile([128, 2], f32, tag="Tb")
                nc.vector.tensor_reduce(
                    Tb[:], wt4[:, 2 * b: 2 * b + 2, :], axis=AxX, op=Alu.add,
                    apply_absolute_value=True,
                )
                Sb = sp.tile([128, 2], f32, tag="Sb")
                nc.vector.tensor_reduce(
                    Sb[:], wt4[:, 2 * b: 2 * b + 2, :], axis=AxX, op=Alu.add
                )
                Sh = sp.tile([128, 2], f32, tag="Sh")
                nc.gpsimd.tensor_scalar(Sh[:], Sb[:], 0.5, None, Alu.mult)

                G = sp.tile([3, A], f16, tag="G")
                for t in range(2):
                    Cc = sp.tile([128, 3], f16, tag="Cc")
                    nc.gpsimd.tensor_copy(
                        Cc[:, 0:1], csc[:, 2 * b + t: 2 * b + t + 1]
                    )
                    nc.vector.scalar_tensor_tensor(
                        Cc[:, 1:2], Tb[:, t: t + 1], 0.5, Sh[:, t: t + 1],
                        Alu.mult, Alu.add,
                    )
                    nc.vector.scalar_tensor_tensor(
                        Cc[:, 2:3], Tb[:, t: t + 1], 0.5, Sh[:, t: t + 1],
                        Alu.mult, Alu.subtract,
                    )
                    tpc = pmm.tile([3, 128], f16, tag="tp1")
                    nc.tensor.transpose(tpc[:], Cc[:], ident[:])
                    nc.vector.tensor_copy(
                        G[:, t * 128: (t + 1) * 128], tpc[:]
                    )

                ps_base = pbase.tile([U, A], f32, tag="psbase")
                nc.tensor.matmul(ps_base[:], F[:], G[:])

                wt = wt4[:, 2 * b: 2 * b + 2, :]
                adjT = sp.tile([128, 2, A], f32, tag="adjT")
                for at in range(2):
                    for jt in range(2):
                        ptr = pmm.tile([128, 128], f32, tag="tp1")
                        nc.tensor.transpose(
                            ptr[:], wt[:, at, jt * 128: (jt + 1) * 128],
                            identf[:],
                        )
                        nc.vector.tensor_scalar(
                            adjT[:, jt, at * 128: (at + 1) * 128],
                            ptr[:], 0.0, None, Alu.not_equal,
                        )
                embT = sp.tile([U, A], f32, tag="embT")
                nc.vector.tensor_scalar(
                    embT[:], ps_base[:], 0.0, None, Alu.max
                )
                EMBb = None
                for it in range(2):
                    nat = sp.tile([128, 2, U], f32, tag="nat")
                    for ht in range(2):
                        pnat = pmm.tile([128, U], f32, tag="tp1")
                        nc.tensor.transpose(
                            pnat[:], embT[:, ht * 128: (ht + 1) * 128],
                            identf[0:U, 0:U],
                        )
                        nc.vector.tensor_copy(nat[:, ht, :], pnat[:])
                    ps_y = pmm.tile([U, A], f32, tag="tp1")
                    nc.tensor.matmul(ps_y[:], nat[:, 0, :], adjT[:, 0, :],
                                     start=True, stop=False)
                    nc.tensor.matmul(ps_y[:], nat[:, 1, :], adjT[:, 1, :],
                                     start=False, stop=True)
                    ysb = sp.tile([U, A], f32, tag="ysb")
                    nc.vector.tensor_scalar(ysb[:], ps_y[:], INV_A, None,
                                            Alu.mult)
                    ps_it = pbase.tile([U, A], f32, tag="psbase")
                    nc.tensor.matmul(ps_it[:], F[:], G[:],
                                     start=True, stop=False)
                    nc.tensor.matmul(ps_it[:], wp[:, WP_W1:WP_W1 + 64],
                                     ysb[:], start=False, stop=True)
                    if it == 0:
                        embT = sp.tile([U, A], f32, tag="embT")
                        nc.vector.tensor_scalar(
                            embT[:], ps_it[:], 0.0, None, Alu.max
                        )
                    else:
                        EMBb = sp.tile([U, A], f16, tag="EMBb")
                        nc.vector.tensor_scalar(
                            EMBb[:], ps_it[:], 0.0, None, Alu.max
                        )

                sl = slice(b * A, (b + 1) * A)
                ph1 = phead.tile([HID, A], f32, tag="pmat")
                nc.tensor.matmul(ph1[:], aw1h, EMBb[:])
                h1 = sp.tile([HID, A], f16, tag="h1")
                nc.scalar.activation(h1[:], ph1[:], Act.Relu,
                                     bias=wp[:, WP_AB1:WP_AB1 + 1])
                ph2 = phead.tile([HID // 2, A], f32, tag="pmat")
                nc.tensor.matmul(ph2[:], aw2h, h1[:])
                h2 = sp.tile([HID // 2, A], f16, tag="h2")
                nc.vector.tensor_scalar(
                    h2[:], ph2[:], wp[0:32, WP_AB2:WP_AB2 + 1], 0.0,
                    Alu.add, op1=Alu.max,
                )
                pa = phead.tile([1, A], f32, tag="pa")
                nc.tensor.matmul(pa[:], aw3h, h2[:])

                phv = phead.tile([HID, A], f32, tag="pmat")
                nc.tensor.matmul(phv[:], vw1h, EMBb[:])
                hv = sp.tile([HID, A], f32, tag="hv")
                hv_cs = sp.tile([U, 1], f32, tag="hv_cs")
                nc.scalar.activation(hv[:], phv[:], Act.Relu,
                                     bias=wp[:, WP_VB1:WP_VB1 + 1],
                                     accum_out=hv_cs[:])
                psv = phead.tile([1, 1], f32, tag="pa")
                nc.tensor.matmul(psv[:], hv_cs[:], wp[:, WP_VW2:WP_VW2 + 1])

                ra = sp.tile([1, 1], f32, tag="ra")
                nc.vector.tensor_reduce(ra[:], pa[:], axis=AxX, op=Alu.add)
                Kb = sp.tile([1, 1], f32, tag="Kb")
                nc.vector.tensor_scalar(
                    Kb[:], psv[:], ra[:], kc[:], Alu.subtract, op1=Alu.add
                )
                nc.vector.scalar_tensor_tensor(
                    FIN[:, sl], pa[:], Kb[:], m10[:, sl], Alu.add, Alu.add
                )
                if b == 0:
                    nc.sync.dma_start(out[b, :][None, :], FIN[:, sl])
                else:
                    nc.scalar.dma_start(out[b, :][None, :], FIN[:, sl])

    return nc


_NC_CACHE: dict[bool, bass.Bass] = {}


def _get_nc(fast: bool) -> bass.Bass:
    if fast not in _NC_CACHE:
        nc = _build_fast() if fast else _build_general()
        nc.finalize()
        _NC_CACHE[fast] = nc
    return _NC_CACHE[fast]


def _make_in_maps(inputs: dict, fast: bool) -> list[dict]:
    x = np.ascontiguousarray(np.asarray(inputs["x"], dtype=np.float32))
    wd = {k: np.asarray(inputs[k], dtype=np.float32) for k in WEIGHT_NAMES}
    wname = "wp2" if fast else "wpack"
    wpk = _pack_weights_fast(wd) if fast else _pack_weights(wd)
    in_maps = []
    for c in range(NCORES):
        in_maps.append({
            "xs": np.ascontiguousarray(x[c * BPC: (c + 1) * BPC]),
            wname: wpk,
        })
    return in_maps


def run(inputs: dict, trace: bool = False, tmpdir: str | None = None):
    """Returns (output [B, A] f32, BassKernelResults)."""
    x = np.asarray(inputs["x"])
    fast = bool((x[:, 2:, :] != 0.0).all())
    nc = _get_nc(fast)
    res = run_bass_kernel_spmd(
        nc, _make_in_maps(inputs, fast), list(range(NCORES)),
        trace=trace, tmpdir=tmpdir,
    )
    out = np.concatenate([res.results[i]["out"] for i in range(NCORES)], axis=0)
    return out, res


def kernel(**inputs) -> np.ndarray:
    out, _ = run(inputs)
    return out
